# revision 28
# baseline (speedup 1.0000x reference)
"""Trainium2 Bass kernel for the MarkovTransitionModule problem.

Strategy:
  - Data-parallel over batch: B=32 sequences -> 4 sequences (4096 tokens) per
    NeuronCore, weights replicated (all small).
  - All on-device activations live feature-major ([features on partitions,
    tokens on free dim]) so no transposes are ever needed on device; the host
    feeds x^T per core and transposes the two outputs back.
  - bf16 matmul inputs (4x faster than fp32 on the PE), fp32 PSUM accumulate,
    fp32 outputs.
  - Softmax over the state dim (which sits on partitions) is done with an
    all-ones matmul that both sums exp() over the partition group and
    broadcasts the denominator back to every partition, then DVE
    reciprocal+multiply.  The transition softmax (groups of 32 partitions)
    uses a block-diagonal ones matrix the same way.
"""

import sys
for _p in ("/opt/trn_rl_repo", "/root/.axon_site/_ro/trn_rl_repo"):
    if _p not in sys.path:
        sys.path.insert(0, _p)

import numpy as np
import ml_dtypes

import concourse.bass as bass
import concourse.bacc as bacc
import concourse.mybir as mybir
import concourse.tile as tile
from concourse.bass_utils import run_bass_kernel_spmd

BF16 = mybir.dt.bfloat16
F32 = mybir.dt.float32
AFT = mybir.ActivationFunctionType
BF16_NP = ml_dtypes.bfloat16
_SENTINEL = object()

# Problem constants (hardcoded per harness contract).
B, L, D, S = 32, 1024, 1024, 32
TAU = 1.0
N_CORES = 8
B_CORE = B // N_CORES            # sequences per core
T_CORE = B_CORE * L              # tokens per core
TILE_T = 512                     # tokens per device tile
KC = D // 128                    # 8 contraction chunks of 128
MC = D // 128                    # 8 output chunks of 128

def build_nc(t_core: int = T_CORE) -> bass.Bass:
    """Build the per-core SPMD Bass program for t_core tokens."""
    assert t_core % TILE_T == 0
    n_tiles = t_core // TILE_T

    nc = bacc.Bacc("TRN2", target_bir_lowering=False, debug=False)

    xT_d = nc.dram_tensor("xT", [D, t_core], BF16, kind="ExternalInput").ap()
    w1_d = nc.dram_tensor("w1", [D, D], BF16, kind="ExternalInput").ap()
    w2_d = nc.dram_tensor("w2", [D, S], BF16, kind="ExternalInput").ap()
    tw1_d = nc.dram_tensor("tw1", [S, 2 * S], BF16, kind="ExternalInput").ap()
    # tw2 / dw1 / bc are shipped pre-replicated along partitions so their
    # matmuls can be row-packed into the PE array (2x / 4x concurrency).
    tw2_d = nc.dram_tensor("tw2", [128, S * S], BF16, kind="ExternalInput").ap()
    dw1_d = nc.dram_tensor("dw1", [128, D], BF16, kind="ExternalInput").ap()
    dw2_d = nc.dram_tensor("dw2", [D, D], BF16, kind="ExternalInput").ap()
    b1_d = nc.dram_tensor("b1", [128, KC], F32, kind="ExternalInput").ap()
    b2_d = nc.dram_tensor("b2", [S, 1], F32, kind="ExternalInput").ap()
    tb1_d = nc.dram_tensor("tb1", [2 * S, 1], F32, kind="ExternalInput").ap()
    tb2_d = nc.dram_tensor("tb2", [128, KC], F32, kind="ExternalInput").ap()
    db1_d = nc.dram_tensor("db1", [128, KC], F32, kind="ExternalInput").ap()
    db2_d = nc.dram_tensor("db2", [128, KC], F32, kind="ExternalInput").ap()
    ones_d = nc.dram_tensor("ones32", [S, S], BF16, kind="ExternalInput").ap()
    # per-chunk group-sum indicators [128, 8*32] and broadcast masks (4-way
    # row-packed: chunk m's [32,128] mask sits at partitions 32*(m%4))
    ind_d = nc.dram_tensor("ind", [128, KC * S], BF16, kind="ExternalInput").ap()
    bc_d = nc.dram_tensor("bc", [128, KC * 128], BF16, kind="ExternalInput").ap()

    mkT_d = nc.dram_tensor("mkT", [D, t_core], F32, kind="ExternalOutput").ap()
    trT_d = nc.dram_tensor("trT", [S * S, t_core], F32, kind="ExternalOutput").ap()

    with tile.TileContext(nc) as tc:
        _emit(tc, n_tiles, xT_d, w1_d, w2_d, tw1_d, tw2_d, dw1_d, dw2_d,
              b1_d, b2_d, tb1_d, tb2_d, db1_d, db2_d, ind_d, bc_d, ones_d,
              mkT_d, trT_d)
    nc.compile()
    return nc


def _emit(tc, n_tiles, xT_d, w1_d, w2_d, tw1_d, tw2_d, dw1_d, dw2_d,
          b1_d, b2_d, tb1_d, tb2_d, db1_d, db2_d, ind_d, bc_d, ones_d,
          mkT_d, trT_d):
    nc = tc.nc
    import contextlib
    ctx = contextlib.ExitStack()
    with ctx:
        wpool = ctx.enter_context(tc.tile_pool(name="weights", bufs=1))
        xpool = ctx.enter_context(tc.tile_pool(name="x", bufs=2))
        hpool = ctx.enter_context(tc.tile_pool(name="h", bufs=2))
        dhpool = ctx.enter_context(tc.tile_pool(name="dh", bufs=2))
        spool = ctx.enter_context(tc.tile_pool(name="small", bufs=3))
        epool = ctx.enter_context(tc.tile_pool(name="etl", bufs=2))
        opool = ctx.enter_context(tc.tile_pool(name="out", bufs=4))
        pspool = ctx.enter_context(tc.tile_pool(name="ps", bufs=8, space="PSUM"))

        # --- resident weights, emitted in first-use order so tile 0 can
        # start as soon as w1 + x(0) land (dw2 isn't needed until ~40us in).
        w1_s = wpool.tile([128, KC * D], BF16)          # [k][m] at (k*MC+m)*128
        w2_s = wpool.tile([128, KC * S], BF16)          # [k] at k*S
        tw1_s = wpool.tile([S, 2 * S], BF16)
        tw2_s = wpool.tile([128, S * S], BF16)
        dw1_s = wpool.tile([128, D], BF16)
        ind_s = wpool.tile([128, KC * S], BF16)
        bc_s = wpool.tile([128, KC * 128], BF16)
        ones_s = wpool.tile([S, S], BF16)
        b1_s = wpool.tile([128, KC], F32)
        b2_s = wpool.tile([S, 1], F32)
        tb1_s = wpool.tile([2 * S, 1], F32)
        tb2_s = wpool.tile([128, KC], F32)
        db1_s = wpool.tile([128, KC], F32)
        db2_s = wpool.tile([128, KC], F32)
        dw2_s = wpool.tile([128, KC * D], BF16)

        def load_weights():
            for k in range(KC):
                nc.sync.dma_start(w1_s[:, k * D:(k + 1) * D],
                                  w1_d[k * 128:(k + 1) * 128, :])
            nc.sync.dma_start(b1_s[:], b1_d[:])
            for k in range(KC):
                nc.sync.dma_start(w2_s[:, k * S:(k + 1) * S],
                                  w2_d[k * 128:(k + 1) * 128, :])
            nc.sync.dma_start(b2_s[:], b2_d[:])
            nc.sync.dma_start(ones_s[:], ones_d[:])
            nc.sync.dma_start(tw1_s[:], tw1_d[:])
            nc.sync.dma_start(tb1_s[:], tb1_d[:])
            nc.sync.dma_start(tw2_s[:], tw2_d[:])
            nc.sync.dma_start(tb2_s[:], tb2_d[:])
            nc.sync.dma_start(dw1_s[:], dw1_d[:])
            nc.sync.dma_start(db1_s[:], db1_d[:])
            nc.sync.dma_start(ind_s[:], ind_d[:])
            nc.sync.dma_start(bc_s[:], bc_d[:])
            nc.sync.dma_start(db2_s[:], db2_d[:])
            for k in range(KC):
                nc.sync.dma_start(dw2_s[:, k * D:(k + 1) * D],
                                  dw2_d[k * 128:(k + 1) * 128, :])

        inv_tau = 1.0 / TAU

        def load_x(t):
            ot = t * TILE_T
            xT_s = xpool.tile([128, KC * TILE_T], BF16, tag="x", name=f"xT{t}")
            for k in range(KC):
                nc.sync.dma_start(
                    xT_s[:, k * TILE_T:(k + 1) * TILE_T],
                    xT_d[k * 128:(k + 1) * 128, ot:ot + TILE_T],
                )
            return xT_s

        def gen_enc1(t, xT_s, h_s):
            """enc layer 1 as a generator: yields after every matmul so the
            driver can use these stall-free MMs as PE filler."""
            for m in range(MC):
                ps = pspool.tile([128, TILE_T], F32, tag="ps", name=f"enc1ps{t}_{m}")
                for k in range(KC):
                    nc.tensor.matmul(
                        ps[:],
                        w1_s[:, (k * MC + m) * 128:(k * MC + m + 1) * 128],
                        xT_s[:, k * TILE_T:(k + 1) * TILE_T],
                        start=(k == 0), stop=(k == KC - 1),
                    )
                    yield
                nc.scalar.activation(
                    h_s[:, m * TILE_T:(m + 1) * TILE_T], ps[:],
                    AFT.Relu, bias=b1_s[:, m:m + 1],
                )

        def gen_middle(t, h_s, dh_s):
            """Everything between enc1 and dec2.  Yields an int N at each
            point where the PE would stall for a dependency: the driver
            inserts ~N filler matmuls from the next tile's enc1."""
            ot = t * TILE_T
            # enc2 (h fully available in steady state - no stalls)
            psl = pspool.tile([S, TILE_T], F32, tag="ps", name=f"psl{t}")
            for k in range(KC):
                nc.tensor.matmul(
                    psl[:],
                    w2_s[:, k * S:(k + 1) * S],
                    h_s[:, k * TILE_T:(k + 1) * TILE_T],
                    start=(k == 0), stop=(k == KC - 1),
                )
            exp_l = spool.tile([S, TILE_T], BF16, tag="expl")
            nc.scalar.activation(exp_l[:], psl[:], AFT.Exp,
                                 bias=b2_s[:], scale=inv_tau)
            yield 4          # wait for exp_l (ACT)
            psd = pspool.tile([S, TILE_T], F32, tag="ps", name=f"psd{t}")
            nc.tensor.matmul(psd[:], ones_s[:], exp_l[:], start=True, stop=True)
            rec = spool.tile([S, TILE_T], F32, tag="rec")
            nc.vector.reciprocal_approx_fast(rec[:], psd[:])
            # probs replicated to 4 partition groups for row-packed matmuls
            probs = spool.tile([128, TILE_T], BF16, tag="probs")
            nc.vector.tensor_mul(probs[0:S, :], exp_l[:], rec[:])
            for i in range(1, 4):
                nc.sync.dma_start(probs[i * S:(i + 1) * S, :], probs[0:S, :])
            yield 8          # wait for rec + probs (DVE)
            pst = pspool.tile([2 * S, TILE_T], F32, tag="ps", name=f"pst{t}")
            nc.tensor.matmul(pst[:], tw1_s[:], probs[0:S, :],
                             start=True, stop=True)
            th = spool.tile([128, TILE_T], BF16, tag="th")
            nc.scalar.activation(th[0:2 * S, :], pst[:], AFT.Relu,
                                 bias=tb1_s[:])
            nc.sync.dma_start(th[2 * S:, :], th[0:2 * S, :])
            yield 4          # wait for th (ACT)
            # transition layer 2 + exp (2-way row-packed: K=64)
            etls = []
            for m in range(MC):
                i = m % 2
                ptl = pspool.tile([128, TILE_T], F32, tag="ps", name=f"ptl{t}_{m}")
                nc.tensor.matmul(
                    ptl[:], tw2_s[64 * i:64 * (i + 1), m * 128:(m + 1) * 128],
                    th[64 * i:64 * (i + 1), :],
                    start=True, stop=True, tile_position=(64 * i, 0),
                )
                etl = epool.tile([128, TILE_T], BF16, tag=f"etl{m}")
                nc.scalar.activation(etl[:], ptl[:], AFT.Exp,
                                     bias=tb2_s[:, m:m + 1], scale=inv_tau)
                etls.append(etl)
            # compact group-sum accumulation over the 8 exp chunks
            ptd = pspool.tile([S, TILE_T], F32, tag="ps", name=f"ptd{t}")
            for m in range(MC):
                nc.tensor.matmul(
                    ptd[:], ind_s[:, m * S:(m + 1) * S], etls[m][:],
                    start=(m == 0), stop=(m == MC - 1), skip_group_check=True,
                )
                if m % 2 == 1:
                    yield 2      # pace behind the etl ACT chain
            rtd = spool.tile([S, TILE_T], F32, tag="rtd")
            nc.vector.reciprocal_approx_fast(rtd[:], ptd[:])
            rec_bf = spool.tile([128, TILE_T], BF16, tag="recbf")
            nc.scalar.activation(rec_bf[0:S, :], rtd[:], AFT.Copy)
            for i in range(1, 4):
                nc.sync.dma_start(rec_bf[i * S:(i + 1) * S, :], rec_bf[0:S, :])
            # decoder layer 1 (4-way row-packed; needs only probs)
            for m in range(MC):
                i = m % 4
                psh = pspool.tile([128, TILE_T], F32, tag="ps", name=f"psh{t}_{m}")
                nc.tensor.matmul(
                    psh[:], dw1_s[S * i:S * (i + 1), m * 128:(m + 1) * 128],
                    probs[S * i:S * (i + 1), :],
                    start=True, stop=True, tile_position=(S * i, 0),
                )
                nc.scalar.activation(
                    dh_s[:, m * TILE_T:(m + 1) * TILE_T], psh[:],
                    AFT.Relu, bias=db1_s[:, m:m + 1],
                )
            # broadcast reciprocals (4-way row-packed), normalize, store
            for m in range(MC):
                i = m % 4
                psr = pspool.tile([128, TILE_T], F32, tag="ps", name=f"psr{t}_{m}")
                nc.tensor.matmul(
                    psr[:], bc_s[S * i:S * (i + 1), m * 128:(m + 1) * 128],
                    rec_bf[S * i:S * (i + 1), :],
                    start=True, stop=True, tile_position=(S * i, 0),
                )
                tro = opool.tile([128, TILE_T], F32, tag="tro")
                nc.vector.tensor_mul(tro[:], etls[m][:], psr[:])
                nc.sync.dma_start(
                    trT_d[m * 128:(m + 1) * 128, ot:ot + TILE_T], tro[:]
                )

        def emit_dec2(t, dh_s):
            # m-outer: each pso[m] finishes early so its mko (ACT) and store
            # overlap the remaining matmuls instead of piling up at the end.
            ot = t * TILE_T
            for m in range(MC):
                pso = pspool.tile([128, TILE_T], F32, tag="ps", name=f"pso{t}_{m}")
                for k in range(KC):
                    nc.tensor.matmul(
                        pso[:],
                        dw2_s[:, (k * MC + m) * 128:(k * MC + m + 1) * 128],
                        dh_s[:, k * TILE_T:(k + 1) * TILE_T],
                        start=(k == 0), stop=(k == KC - 1),
                    )
                mko = opool.tile([128, TILE_T], F32, tag="mko")
                nc.scalar.activation(mko[:], pso[:], AFT.Identity,
                                     bias=db2_s[:, m:m + 1])
                nc.sync.dma_start(
                    mkT_d[m * 128:(m + 1) * 128, ot:ot + TILE_T], mko[:]
                )

        # --- software pipeline: enc1(t+1) fills middle(t)'s PE stalls ---
        hs = [hpool.tile([128, KC * TILE_T], BF16, tag="h", name=f"h{t}")
              for t in range(n_tiles)]
        dhs = [dhpool.tile([128, KC * TILE_T], BF16, tag="dh", name=f"dh{t}")
               for t in range(n_tiles)]

        xT0 = load_x(0)
        load_weights()
        for _ in gen_enc1(0, xT0, hs[0]):
            pass
        for t in range(n_tiles):
            if t + 1 < n_tiles:
                xT1 = load_x(t + 1)
                filler = gen_enc1(t + 1, xT1, hs[t + 1])
            else:
                filler = None
            for need in gen_middle(t, hs[t], dhs[t]):
                if filler is not None:
                    for _ in range(need):
                        if next(filler, _SENTINEL) is _SENTINEL:
                            filler = None
                            break
            if filler is not None:
                for _ in filler:
                    pass
            emit_dec2(t, dhs[t])


_NC_CACHE: dict[int, bass.Bass] = {}
TRACE = False            # set True (e.g. from test.py) to capture an NTFF trace
LAST_EXEC_NS = None      # filled after each kernel() call when TRACE is on
LAST_RESULTS: dict = {}  # last BassKernelResults, for trace inspection


def _get_nc(t_core: int) -> bass.Bass:
    if t_core not in _NC_CACHE:
        _NC_CACHE[t_core] = build_nc(t_core)
    return _NC_CACHE[t_core]


def _shared_inputs(inputs: dict) -> dict:
    f32 = np.float32

    def bf(a):
        return np.ascontiguousarray(a).astype(BF16_NP)

    def cols(a):  # [KC*128] vector -> [128, KC] column-chunk layout
        return np.ascontiguousarray(a.astype(f32).reshape(KC, 128).T)

    # ind[k, m*32+c] = 1 iff c == 4*m + k//32  (group-sum accumulator)
    ind = np.zeros((128, KC * S), np.float32)
    for m in range(KC):
        for k in range(128):
            ind[k, m * S + 4 * m + k // 32] = 1.0
    # bc4: 4-way row-packed denominator-broadcast masks; chunk m's mask lives
    # at partitions 32*(m%4): bc4[32*(m%4) + (4m + j//32), m*128 + j] = 1
    bc4 = np.zeros((128, KC * 128), np.float32)
    for m in range(KC):
        i = m % 4
        for j in range(128):
            bc4[S * i + 4 * m + j // 32, m * 128 + j] = 1.0
    return {
        "w1": bf(inputs["enc_w1"]),
        "w2": bf(inputs["enc_w2"]),
        "tw1": bf(inputs["tr_w1"]),
        "tw2": bf(np.vstack([inputs["tr_w2"]] * 2)),
        "dw1": bf(np.vstack([inputs["dec_w1"]] * 4)),
        "dw2": bf(inputs["dec_w2"]),
        "b1": cols(inputs["enc_b1"]),
        "b2": np.ascontiguousarray(
            (inputs["enc_b2"].astype(f32) / TAU).reshape(S, 1)),
        "tb1": np.ascontiguousarray(inputs["tr_b1"].astype(f32).reshape(2 * S, 1)),
        "tb2": cols(inputs["tr_b2"].astype(f32) / TAU),
        "db1": cols(inputs["dec_b1"]),
        "db2": cols(inputs["dec_b2"]),
        "ind": ind.astype(BF16_NP),
        "bc": bc4.astype(BF16_NP),
        "ones32": np.ones((S, S), BF16_NP),
    }


def kernel(**inputs) -> tuple[np.ndarray, np.ndarray]:
    x = np.asarray(inputs["x"], dtype=np.float32)
    assert x.shape == (B, L, D)

    nc = _get_nc(T_CORE)
    shared = _shared_inputs(inputs)

    in_maps = []
    for c in range(N_CORES):
        xc = x[c * B_CORE:(c + 1) * B_CORE].reshape(T_CORE, D)
        xT = np.ascontiguousarray(xc.T).astype(BF16_NP)
        in_maps.append({**shared, "xT": xT})

    global LAST_EXEC_NS
    res = run_bass_kernel_spmd(nc, in_maps, core_ids=list(range(N_CORES)),
                               trace=TRACE)
    LAST_EXEC_NS = res.exec_time_ns
    LAST_RESULTS[0] = res

    markov = np.empty((B, L, D), np.float32)
    trans = np.empty((B, L - 1, S, S), np.float32)
    for c in range(N_CORES):
        mkT = res.results[c]["mkT"]            # [D, T_CORE]
        markov[c * B_CORE:(c + 1) * B_CORE] = (
            mkT.T.reshape(B_CORE, L, D))
        trT = res.results[c]["trT"]            # [S*S, T_CORE]
        trans[c * B_CORE:(c + 1) * B_CORE] = (
            trT.T.reshape(B_CORE, L, S, S)[:, :L - 1])
    return markov, trans


# revision 33
# speedup vs baseline: 1.0964x; 1.0964x over previous
"""Trainium2 Bass kernel for the MarkovTransitionModule problem.

Strategy:
  - Data-parallel over batch: B=32 sequences -> 4 sequences (4096 tokens) per
    NeuronCore, weights replicated (all small).
  - All on-device activations live feature-major ([features on partitions,
    tokens on free dim]) so no transposes are ever needed on device; the host
    feeds x^T per core and transposes the two outputs back.
  - bf16 matmul inputs (4x faster than fp32 on the PE), fp32 PSUM accumulate,
    fp32 outputs.
  - Softmax over the state dim (which sits on partitions) is done with an
    all-ones matmul that both sums exp() over the partition group and
    broadcasts the denominator back to every partition, then DVE
    reciprocal+multiply.  The transition softmax (groups of 32 partitions)
    uses a block-diagonal ones matrix the same way.
"""

import sys
for _p in ("/opt/trn_rl_repo", "/root/.axon_site/_ro/trn_rl_repo"):
    if _p not in sys.path:
        sys.path.insert(0, _p)

import numpy as np
import ml_dtypes

import concourse.bass as bass
import concourse.bacc as bacc
import concourse.mybir as mybir
import concourse.tile as tile
from concourse.bass_utils import run_bass_kernel_spmd

BF16 = mybir.dt.bfloat16
F32 = mybir.dt.float32
AFT = mybir.ActivationFunctionType
BF16_NP = ml_dtypes.bfloat16
_SENTINEL = object()

# Problem constants (hardcoded per harness contract).
B, L, D, S = 32, 1024, 1024, 32
TAU = 1.0
N_CORES = 8
B_CORE = B // N_CORES            # sequences per core
T_CORE = B_CORE * L              # tokens per core
TILE_T = 512                     # tokens per device tile
KC = D // 128                    # 8 contraction chunks of 128
MC = D // 128                    # 8 output chunks of 128

def build_nc(t_core: int = T_CORE) -> bass.Bass:
    """Build the per-core SPMD Bass program for t_core tokens."""
    assert t_core % TILE_T == 0
    n_tiles = t_core // TILE_T

    nc = bacc.Bacc("TRN2", target_bir_lowering=False, debug=False)

    xT_d = nc.dram_tensor("xT", [D, t_core], BF16, kind="ExternalInput").ap()
    w1_d = nc.dram_tensor("w1", [D, D], BF16, kind="ExternalInput").ap()
    # Several stationary operands are shipped replicated along the output
    # (M) or partition (K) axis so downstream results come out of the PE
    # already replicated across partition groups - which lets the small-K
    # matmuls (tr2 2-way, dec1/broadcast 4-way) row-pack into the array
    # without any SBUF->SBUF replication DMAs.
    w2_d = nc.dram_tensor("w2", [D, 128], BF16, kind="ExternalInput").ap()
    tw1_d = nc.dram_tensor("tw1", [S, 128], BF16, kind="ExternalInput").ap()
    tw2_d = nc.dram_tensor("tw2", [128, S * S], BF16, kind="ExternalInput").ap()
    dw1_d = nc.dram_tensor("dw1", [128, D], BF16, kind="ExternalInput").ap()
    dw2_d = nc.dram_tensor("dw2", [D, D], BF16, kind="ExternalInput").ap()
    b1_d = nc.dram_tensor("b1", [128, KC], F32, kind="ExternalInput").ap()
    b2_d = nc.dram_tensor("b2", [128, 1], F32, kind="ExternalInput").ap()
    tb1_d = nc.dram_tensor("tb1", [128, 1], F32, kind="ExternalInput").ap()
    tb2_d = nc.dram_tensor("tb2", [128, KC], F32, kind="ExternalInput").ap()
    db1_d = nc.dram_tensor("db1", [128, KC], F32, kind="ExternalInput").ap()
    db2_d = nc.dram_tensor("db2", [128, KC], F32, kind="ExternalInput").ap()
    ones_d = nc.dram_tensor("ones32", [S, 128], BF16, kind="ExternalInput").ap()
    ind_d = nc.dram_tensor("ind", [128, KC * 128], BF16, kind="ExternalInput").ap()
    bc_d = nc.dram_tensor("bc", [128, KC * 128], BF16, kind="ExternalInput").ap()

    mkT_d = nc.dram_tensor("mkT", [D, t_core], F32, kind="ExternalOutput").ap()
    trT_d = nc.dram_tensor("trT", [S * S, t_core], F32, kind="ExternalOutput").ap()

    with tile.TileContext(nc) as tc:
        _emit(tc, n_tiles, xT_d, w1_d, w2_d, tw1_d, tw2_d, dw1_d, dw2_d,
              b1_d, b2_d, tb1_d, tb2_d, db1_d, db2_d, ind_d, bc_d, ones_d,
              mkT_d, trT_d)
    nc.compile()
    return nc


def _emit(tc, n_tiles, xT_d, w1_d, w2_d, tw1_d, tw2_d, dw1_d, dw2_d,
          b1_d, b2_d, tb1_d, tb2_d, db1_d, db2_d, ind_d, bc_d, ones_d,
          mkT_d, trT_d):
    nc = tc.nc
    import contextlib
    ctx = contextlib.ExitStack()
    with ctx:
        wpool = ctx.enter_context(tc.tile_pool(name="weights", bufs=1))
        xpool = ctx.enter_context(tc.tile_pool(name="x", bufs=2))
        hpool = ctx.enter_context(tc.tile_pool(name="h", bufs=2))
        dhpool = ctx.enter_context(tc.tile_pool(name="dh", bufs=2))
        spool = ctx.enter_context(tc.tile_pool(name="small", bufs=3))
        epool = ctx.enter_context(tc.tile_pool(name="etl", bufs=2))
        opool = ctx.enter_context(tc.tile_pool(name="out", bufs=4))
        pspool = ctx.enter_context(tc.tile_pool(name="ps", bufs=8, space="PSUM"))

        # --- resident weights, emitted in first-use order so tile 0 can
        # start as soon as w1 + x(0) land (dw2 isn't needed until ~40us in).
        w1_s = wpool.tile([128, KC * D], BF16)          # [k][m] at (k*MC+m)*128
        w2_s = wpool.tile([128, KC * 128], BF16)        # [k] at k*128
        tw1_s = wpool.tile([S, 128], BF16)
        tw2_s = wpool.tile([128, S * S], BF16)
        dw1_s = wpool.tile([128, D], BF16)
        ind_s = wpool.tile([128, KC * 128], BF16)
        bc_s = wpool.tile([128, KC * 128], BF16)
        ones_s = wpool.tile([S, 128], BF16)
        b1_s = wpool.tile([128, KC], F32)
        b2_s = wpool.tile([128, 1], F32)
        tb1_s = wpool.tile([128, 1], F32)
        tb2_s = wpool.tile([128, KC], F32)
        db1_s = wpool.tile([128, KC], F32)
        db2_s = wpool.tile([128, KC], F32)
        dw2_s = wpool.tile([128, KC * D], BF16)

        def load_weights():
            for k in range(KC):
                nc.sync.dma_start(w1_s[:, k * D:(k + 1) * D],
                                  w1_d[k * 128:(k + 1) * 128, :])
            nc.sync.dma_start(b1_s[:], b1_d[:])
            for k in range(KC):
                nc.sync.dma_start(w2_s[:, k * 128:(k + 1) * 128],
                                  w2_d[k * 128:(k + 1) * 128, :])
            nc.sync.dma_start(b2_s[:], b2_d[:])
            nc.sync.dma_start(ones_s[:], ones_d[:])
            nc.sync.dma_start(tw1_s[:], tw1_d[:])
            nc.sync.dma_start(tb1_s[:], tb1_d[:])
            nc.sync.dma_start(tw2_s[:], tw2_d[:])
            nc.sync.dma_start(tb2_s[:], tb2_d[:])
            nc.sync.dma_start(dw1_s[:], dw1_d[:])
            nc.sync.dma_start(db1_s[:], db1_d[:])
            nc.sync.dma_start(ind_s[:], ind_d[:])
            nc.sync.dma_start(bc_s[:], bc_d[:])
            nc.sync.dma_start(db2_s[:], db2_d[:])
            for k in range(KC):
                nc.sync.dma_start(dw2_s[:, k * D:(k + 1) * D],
                                  dw2_d[k * 128:(k + 1) * 128, :])

        inv_tau = 1.0 / TAU

        def load_x(t):
            ot = t * TILE_T
            xT_s = xpool.tile([128, KC * TILE_T], BF16, tag="x", name=f"xT{t}")
            for k in range(KC):
                nc.sync.dma_start(
                    xT_s[:, k * TILE_T:(k + 1) * TILE_T],
                    xT_d[k * 128:(k + 1) * 128, ot:ot + TILE_T],
                )
            return xT_s

        def gen_enc1(t, xT_s, h_s):
            """enc layer 1 as a generator: yields after every matmul so the
            driver can use these stall-free MMs as PE filler."""
            for m in range(MC):
                ps = pspool.tile([128, TILE_T], F32, tag="ps", name=f"enc1ps{t}_{m}")
                for k in range(KC):
                    nc.tensor.matmul(
                        ps[:],
                        w1_s[:, (k * MC + m) * 128:(k * MC + m + 1) * 128],
                        xT_s[:, k * TILE_T:(k + 1) * TILE_T],
                        start=(k == 0), stop=(k == KC - 1),
                    )
                    yield
                nc.scalar.activation(
                    h_s[:, m * TILE_T:(m + 1) * TILE_T], ps[:],
                    AFT.Relu, bias=b1_s[:, m:m + 1],
                )

        def gen_middle(t, h_s, dh_s):
            """Everything between enc1 and dec2.  Yields an int N at each
            point where the PE would stall for a dependency: the driver
            inserts ~N filler matmuls from the next tile's enc1."""
            ot = t * TILE_T
            # enc2 (h fully available in steady state - no stalls).  w2 is
            # shipped M-replicated 4x, so logits come out replicated in all
            # four 32-row groups.
            psl = pspool.tile([128, TILE_T], F32, tag="ps", name=f"psl{t}")
            for k in range(KC):
                nc.tensor.matmul(
                    psl[:],
                    w2_s[:, k * 128:(k + 1) * 128],
                    h_s[:, k * TILE_T:(k + 1) * TILE_T],
                    start=(k == 0), stop=(k == KC - 1),
                )
            exp_l = spool.tile([128, TILE_T], BF16, tag="expl")
            nc.scalar.activation(exp_l[:], psl[:], AFT.Exp,
                                 bias=b2_s[:], scale=inv_tau)
            yield 4          # wait for exp_l (ACT)
            # all-ones [32,128] lhsT -> denominator replicated on all rows
            psd = pspool.tile([128, TILE_T], F32, tag="ps", name=f"psd{t}")
            nc.tensor.matmul(psd[:], ones_s[:], exp_l[0:S, :],
                             start=True, stop=True)
            rec = spool.tile([128, TILE_T], F32, tag="rec")
            nc.vector.reciprocal_approx_fast(rec[:], psd[:])
            probs = spool.tile([128, TILE_T], BF16, tag="probs")
            nc.vector.tensor_mul(probs[:], exp_l[:], rec[:])
            yield 8          # wait for rec + probs (DVE)
            # tw1 shipped M-replicated 2x -> th replicated in both 64-halves
            pst = pspool.tile([128, TILE_T], F32, tag="ps", name=f"pst{t}")
            nc.tensor.matmul(pst[:], tw1_s[:], probs[0:S, :],
                             start=True, stop=True)
            th = spool.tile([128, TILE_T], BF16, tag="th")
            nc.scalar.activation(th[:], pst[:], AFT.Relu, bias=tb1_s[:])
            yield 4          # wait for th (ACT)
            # transition layer 2 + exp (2-way row-packed: K=64)
            etls = []
            for m in range(MC):
                i = m % 2
                ptl = pspool.tile([128, TILE_T], F32, tag="ps", name=f"ptl{t}_{m}")
                nc.tensor.matmul(
                    ptl[:], tw2_s[64 * i:64 * (i + 1), m * 128:(m + 1) * 128],
                    th[64 * i:64 * (i + 1), :],
                    start=True, stop=True, tile_position=(64 * i, 0),
                )
                etl = epool.tile([128, TILE_T], BF16, tag=f"etl{m}")
                nc.scalar.activation(etl[:], ptl[:], AFT.Exp,
                                     bias=tb2_s[:, m:m + 1], scale=inv_tau)
                etls.append(etl)
            # compact group-sum accumulation over the 8 exp chunks; the ind
            # masks write each chunk's 4 group-sums into all four 32-row
            # replicas, so the reciprocals come out pre-replicated.
            ptd = pspool.tile([128, TILE_T], F32, tag="ps", name=f"ptd{t}")
            for m in range(MC):
                nc.tensor.matmul(
                    ptd[:], ind_s[:, m * 128:(m + 1) * 128], etls[m][:],
                    start=(m == 0), stop=(m == MC - 1), skip_group_check=True,
                )
                if m % 2 == 1:
                    yield 2      # pace behind the etl ACT chain
            rtd = spool.tile([128, TILE_T], F32, tag="rtd")
            nc.vector.reciprocal_approx_fast(rtd[:], ptd[:])
            rec_bf = spool.tile([128, TILE_T], BF16, tag="recbf")
            nc.scalar.activation(rec_bf[:], rtd[:], AFT.Copy)
            # decoder layer 1 (4-way row-packed; needs only probs)
            for m in range(MC):
                i = m % 4
                psh = pspool.tile([128, TILE_T], F32, tag="ps", name=f"psh{t}_{m}")
                nc.tensor.matmul(
                    psh[:], dw1_s[S * i:S * (i + 1), m * 128:(m + 1) * 128],
                    probs[S * i:S * (i + 1), :],
                    start=True, stop=True, tile_position=(S * i, 0),
                )
                nc.scalar.activation(
                    dh_s[:, m * TILE_T:(m + 1) * TILE_T], psh[:],
                    AFT.Relu, bias=db1_s[:, m:m + 1],
                )
            # broadcast reciprocals (4-way row-packed), normalize, store
            for m in range(MC):
                i = m % 4
                psr = pspool.tile([128, TILE_T], F32, tag="ps", name=f"psr{t}_{m}")
                nc.tensor.matmul(
                    psr[:], bc_s[S * i:S * (i + 1), m * 128:(m + 1) * 128],
                    rec_bf[S * i:S * (i + 1), :],
                    start=True, stop=True, tile_position=(S * i, 0),
                )
                tro = opool.tile([128, TILE_T], F32, tag="tro")
                nc.vector.tensor_mul(tro[:], etls[m][:], psr[:])
                nc.sync.dma_start(
                    trT_d[m * 128:(m + 1) * 128, ot:ot + TILE_T], tro[:]
                )

        def emit_dec2(t, dh_s):
            # m-outer: each pso[m] finishes early so its mko (ACT) and store
            # overlap the remaining matmuls instead of piling up at the end.
            ot = t * TILE_T
            for m in range(MC):
                pso = pspool.tile([128, TILE_T], F32, tag="ps", name=f"pso{t}_{m}")
                for k in range(KC):
                    nc.tensor.matmul(
                        pso[:],
                        dw2_s[:, (k * MC + m) * 128:(k * MC + m + 1) * 128],
                        dh_s[:, k * TILE_T:(k + 1) * TILE_T],
                        start=(k == 0), stop=(k == KC - 1),
                    )
                mko = opool.tile([128, TILE_T], F32, tag="mko")
                nc.scalar.activation(mko[:], pso[:], AFT.Identity,
                                     bias=db2_s[:, m:m + 1])
                nc.sync.dma_start(
                    mkT_d[m * 128:(m + 1) * 128, ot:ot + TILE_T], mko[:]
                )

        # --- software pipeline: enc1(t+1) fills middle(t)'s PE stalls ---
        hs = [hpool.tile([128, KC * TILE_T], BF16, tag="h", name=f"h{t}")
              for t in range(n_tiles)]
        dhs = [dhpool.tile([128, KC * TILE_T], BF16, tag="dh", name=f"dh{t}")
               for t in range(n_tiles)]

        xT0 = load_x(0)
        load_weights()
        for _ in gen_enc1(0, xT0, hs[0]):
            pass
        for t in range(n_tiles):
            if t + 1 < n_tiles:
                xT1 = load_x(t + 1)
                filler = gen_enc1(t + 1, xT1, hs[t + 1])
            else:
                filler = None
            for need in gen_middle(t, hs[t], dhs[t]):
                if filler is not None:
                    for _ in range(need):
                        if next(filler, _SENTINEL) is _SENTINEL:
                            filler = None
                            break
            if filler is not None:
                for _ in filler:
                    pass
            emit_dec2(t, dhs[t])


_NC_CACHE: dict[int, bass.Bass] = {}
TRACE = False            # set True (e.g. from test.py) to capture an NTFF trace
LAST_EXEC_NS = None      # filled after each kernel() call when TRACE is on
LAST_RESULTS: dict = {}  # last BassKernelResults, for trace inspection


def _get_nc(t_core: int) -> bass.Bass:
    if t_core not in _NC_CACHE:
        _NC_CACHE[t_core] = build_nc(t_core)
    return _NC_CACHE[t_core]


def _shared_inputs(inputs: dict) -> dict:
    f32 = np.float32

    def bf(a):
        return np.ascontiguousarray(a).astype(BF16_NP)

    def cols(a):  # [KC*128] vector -> [128, KC] column-chunk layout
        return np.ascontiguousarray(a.astype(f32).reshape(KC, 128).T)

    # ind: group-sum masks, M-replicated 4x so the denominators land
    # replicated: ind[k, m*128 + j] = 1 iff j%32 == 4*m + k//32
    ind = np.zeros((128, KC * 128), np.float32)
    for m in range(KC):
        for k in range(128):
            g = 4 * m + k // 32
            for i in range(4):
                ind[k, m * 128 + S * i + g] = 1.0
    # bc4: 4-way row-packed denominator-broadcast masks; chunk m's mask lives
    # at partitions 32*(m%4): bc4[32*(m%4) + (4m + j//32), m*128 + j] = 1
    bc4 = np.zeros((128, KC * 128), np.float32)
    for m in range(KC):
        i = m % 4
        for j in range(128):
            bc4[S * i + 4 * m + j // 32, m * 128 + j] = 1.0
    return {
        "w1": bf(inputs["enc_w1"]),
        "w2": bf(np.hstack([np.asarray(inputs["enc_w2"])] * 4)),
        "tw1": bf(np.hstack([np.asarray(inputs["tr_w1"])] * 2)),
        "tw2": bf(np.vstack([inputs["tr_w2"]] * 2)),
        "dw1": bf(np.vstack([inputs["dec_w1"]] * 4)),
        "dw2": bf(inputs["dec_w2"]),
        "b1": cols(inputs["enc_b1"]),
        "b2": np.ascontiguousarray(
            np.tile(np.asarray(inputs["enc_b2"]).astype(f32) / TAU, 4)
            .reshape(128, 1)),
        "tb1": np.ascontiguousarray(
            np.tile(np.asarray(inputs["tr_b1"]).astype(f32), 2)
            .reshape(128, 1)),
        "tb2": cols(inputs["tr_b2"].astype(f32) / TAU),
        "db1": cols(inputs["dec_b1"]),
        "db2": cols(inputs["dec_b2"]),
        "ind": ind.astype(BF16_NP),
        "bc": bc4.astype(BF16_NP),
        "ones32": np.ones((S, 128), BF16_NP),
    }


def kernel(**inputs) -> tuple[np.ndarray, np.ndarray]:
    x = np.asarray(inputs["x"], dtype=np.float32)
    assert x.shape == (B, L, D)

    nc = _get_nc(T_CORE)
    shared = _shared_inputs(inputs)

    in_maps = []
    for c in range(N_CORES):
        xc = x[c * B_CORE:(c + 1) * B_CORE].reshape(T_CORE, D)
        xT = np.ascontiguousarray(xc.T).astype(BF16_NP)
        in_maps.append({**shared, "xT": xT})

    global LAST_EXEC_NS
    res = run_bass_kernel_spmd(nc, in_maps, core_ids=list(range(N_CORES)),
                               trace=TRACE)
    LAST_EXEC_NS = res.exec_time_ns
    LAST_RESULTS[0] = res

    markov = np.empty((B, L, D), np.float32)
    trans = np.empty((B, L - 1, S, S), np.float32)
    for c in range(N_CORES):
        mkT = res.results[c]["mkT"]            # [D, T_CORE]
        markov[c * B_CORE:(c + 1) * B_CORE] = (
            mkT.T.reshape(B_CORE, L, D))
        trT = res.results[c]["trT"]            # [S*S, T_CORE]
        trans[c * B_CORE:(c + 1) * B_CORE] = (
            trT.T.reshape(B_CORE, L, S, S)[:, :L - 1])
    return markov, trans
